# revision 10
# baseline (speedup 1.0000x reference)
"""Trainium2 Bass kernel for nn_AdaRRD_Decoder — physical-domain restructure.

Strategy
--------
Data-parallel over batch: B=128 split 16 per core.  All node-space tensors are
keyed by PHYSICAL variable id, which folds the per-tau automorphism perms into
the (compile-time known) edge routing tables:
  - soft mixing, output, and the tau->tau carry become pure elementwise ops;
  - per inner iteration only TWO irregular gathers remain:
      tk : gamma-scaled node tensor t -> check-sorted edge positions
      cv : damped C2V messages (check-sorted) -> var-sorted edge positions
    Each is 16 dma_gather chunks spread round-robin over 4 SWDGE queues
    (parallel descriptor rings ~3x the single-ring drain rate).
Check-side leave-one-out sums are (J8-I8) block-diag matmuls on TensorE in the
check-sorted layout; var-side column sums are fixed rmat matmuls in the
var-sorted layout.  Transcendentals use only Exp/Ln (one ACT table set):
  log(tanh(x/2)) = log(1-e^-x) - log(1+e^-x)
  2*atanh(y)     = log(1+y) - log(1-y)
"""

import numpy as np

N = 4096
M = 2048
DV = 4
DC = 8
E = N * DV
B_FULL = 128
N_CORES = 8
B_LOC = B_FULL // N_CORES  # 16
T_RRD = 10
T_INNER = 2
LLR_CLIP = 15.0
AMP_MIN = float(-np.log(np.tanh(LLR_CLIP / 2.0)))
TH_MIN = float(np.tanh(-np.log(np.tanh(LLR_CLIP / 2.0)) / 2.0))
EPS1 = 1.0 - 1e-6
LOG10E_10 = float(10.0 / np.log(10.0))

EC = E // 128   # 128 edge chunks
NC_ = N // 128  # 32 var chunks
EF = EC * B_LOC   # 2048
NF = NC_ * B_LOC  # 512
CH = 1024
NQ = 4  # SWDGE queues

_CACHE = {}


def _build_nc():
    import concourse.bass as bass
    import concourse.bacc as bacc
    import concourse.tile as tile
    from concourse import mybir

    f32 = mybir.dt.float32
    bf16 = mybir.dt.bfloat16
    i32 = mybir.dt.int32
    i16 = mybir.dt.int16
    AF = mybir.ActivationFunctionType
    OP = mybir.AluOpType

    nc = bacc.Bacc(trn_type="TRN2", dynamic_dma_scratch_size=32768,
                   num_swdge_queues=NQ)

    # ---- external I/O ----
    chn_h = nc.declare_dram_parameter("chn", [128, NF], f32, isOutput=False)
    w1_h = nc.declare_dram_parameter("W1", [20, 1], f32, isOutput=False)
    b1_h = nc.declare_dram_parameter("b1", [20, 1], f32, isOutput=False)
    w2_h = nc.declare_dram_parameter("W2", [4, 20], f32, isOutput=False)
    b2_h = nc.declare_dram_parameter("b2", [4, 1], f32, isOutput=False)
    EW = E // 16  # 1024 idx cols per tau
    tk_h = nc.declare_dram_parameter("tk16", [128, T_RRD * EW], i16, isOutput=False)
    cv_h = nc.declare_dram_parameter("cv16", [128, T_RRD * EW], i16, isOutput=False)
    chnk_h = nc.declare_dram_parameter(
        "chnk", [128, T_RRD * EF], f32, isOutput=False)
    mloo_h = nc.declare_dram_parameter("mloo", [128, 128], f32, isOutput=False)
    mloon_h = nc.declare_dram_parameter("mloon", [128, 128], f32, isOutput=False)
    rmat_h = nc.declare_dram_parameter("rmat", [128, 512], f32, isOutput=False)
    out_h = nc.declare_dram_parameter(
        "out", [T_RRD * T_INNER * N, B_LOC], f32, isOutput=True)

    # DRAM staging (rows padded to 256 B for gather elem granularity)
    t_dram = nc.dram_tensor("t_dram", [N, 64], f32)
    c2v_dram = nc.dram_tensor("c2v_dram", [E, 128], bf16)

    def bce(coef):
        return coef[:, :].rearrange("p (c b) -> p c b", c=1).to_broadcast(
            [128, EC, B_LOC])

    def bcn(coef):
        return coef[:, :].rearrange("p (c b) -> p c b", c=1).to_broadcast(
            [128, NC_, B_LOC])

    def e3(ap):
        return ap.rearrange("p (c b) -> p c b", b=B_LOC)

    def sel(gdst, w):
        return gdst[:, :].rearrange("p (c w) -> p c w", w=w)[:, :, 0:B_LOC]

    with tile.TileContext(nc) as tc:
        with (
            tc.tile_pool(name="const", bufs=1) as pconst,
            tc.tile_pool(name="work", bufs=1) as pwork,
            tc.tile_pool(name="nwork", bufs=1) as pnw,
            tc.tile_pool(name="psA", bufs=1, space="PSUM") as ppsA,
            tc.tile_pool(name="psB", bufs=1, space="PSUM") as ppsB,
        ):
            # ---------- constants ----------
            tk_sb = pconst.tile([128, T_RRD * EW], i16)
            nc.sync.dma_start(out=tk_sb[:, :], in_=tk_h[:, :])
            cv_sb = pconst.tile([128, T_RRD * EW], i16)
            nc.sync.dma_start(out=cv_sb[:, :], in_=cv_h[:, :])
            # zero padded staging (pad cols are gathered; must be defined)
            zt = pconst.tile([128, 512], f32)
            nc.vector.memset(zt[:, :], 0.0)
            zf = zt[:, :].rearrange("p (a b) -> p a b", b=64)
            ztb = zt[:, :].bitcast(bf16).rearrange("p (a b) -> p a b", b=128)
            for z in range(4):
                nc.sync.dma_start(
                    out=t_dram[z * N // 4:(z + 1) * N // 4, :].rearrange(
                        "(p a) b -> p a b", p=128), in_=zf)
            for z in range(16):
                nc.sync.dma_start(
                    out=c2v_dram[z * E // 16:(z + 1) * E // 16, :].rearrange(
                        "(p a) b -> p a b", p=128), in_=ztb)
            mloo_sb = pconst.tile([128, 128], f32)
            nc.sync.dma_start(out=mloo_sb[:, :], in_=mloo_h[:, :])
            mloon_sb = pconst.tile([128, 128], f32)
            nc.sync.dma_start(out=mloon_sb[:, :], in_=mloon_h[:, :])
            rmat_sb = pconst.tile([128, 512], f32)
            nc.sync.dma_start(out=rmat_sb[:, :], in_=rmat_h[:, :])
            mloo_bf = pconst.tile([128, 128], bf16)
            nc.vector.tensor_copy(mloo_bf[:, :], mloo_sb[:, :])
            mloon_bf = pconst.tile([128, 128], bf16)
            nc.vector.tensor_copy(mloon_bf[:, :], mloon_sb[:, :])
            rmat_bf = pconst.tile([128, 512], bf16)
            nc.vector.tensor_copy(rmat_bf[:, :], rmat_sb[:, :])

            chn_sb = pconst.tile([128, NF], f32)
            nc.sync.dma_start(out=chn_sb[:, :], in_=chn_h[:, :])

            # ---------- adapter NN ----------
            sq = pnw.tile([128, NF], f32, tag="nw")
            nc.scalar.activation(sq[:, :], chn_sb[:, :], AF.Square)
            ones128 = pconst.tile([128, 1], f32)
            nc.vector.memset(ones128[:, :], 1.0)
            e1_ps = ppsB.tile([1, NF], f32, tag="aux")
            nc.tensor.matmul(out=e1_ps[:, :], lhsT=ones128[:, :],
                             rhs=sq[:, :], start=True, stop=True)
            e1_sb = pconst.tile([1, NF], f32)
            nc.vector.tensor_copy(e1_sb[:, :], e1_ps[:, :])
            e32 = pconst.tile([NC_, B_LOC], f32)
            nc.sync.dma_start(
                out=e32[:, :],
                in_=e1_sb[0:1, :].rearrange("x (c b) -> x c b", c=NC_))
            ones32 = pconst.tile([NC_, 1], f32)
            nc.vector.memset(ones32[:, :], 1.0)
            est_ps = ppsB.tile([1, B_LOC], f32, tag="aux")
            nc.tensor.matmul(out=est_ps[:, :], lhsT=ones32[:, :],
                             rhs=e32[:, :], start=True, stop=True)
            est = pconst.tile([1, B_LOC], f32)
            nc.scalar.activation(est[:, :], est_ps[:, :], AF.Copy, scale=1.0 / N)
            s1 = pconst.tile([1, B_LOC], f32)
            nc.scalar.activation(s1[:, :], est_ps[:, :], AF.Ln, scale=1.0 / N, bias=1.0)
            s2 = pconst.tile([1, B_LOC], f32)
            nc.scalar.activation(s2[:, :], s1[:, :], AF.Exp, scale=0.5)
            nc.vector.tensor_scalar(
                out=s2[:, :], in0=s2[:, :], scalar1=1.0, scalar2=None, op0=OP.add)
            nc.vector.reciprocal(s2[:, :], s2[:, :])
            q = pconst.tile([1, B_LOC], f32)
            nc.vector.tensor_tensor(out=q[:, :], in0=est[:, :], in1=s2[:, :], op=OP.mult)
            snrl = pconst.tile([1, B_LOC], f32)
            nc.scalar.activation(snrl[:, :], q[:, :], AF.Ln, scale=0.5)

            w1t = pconst.tile([1, 20], f32)
            nc.sync.dma_start(out=w1t[:, :], in_=w1_h[:, :].rearrange("a b -> b a"))
            nc.scalar.activation(w1t[:, :], w1t[:, :], AF.Copy, scale=LOG10E_10)
            b1s = pconst.tile([20, 1], f32)
            nc.sync.dma_start(out=b1s[:, :], in_=b1_h[:, :])
            h_ps = ppsB.tile([20, B_LOC], f32, tag="aux")
            nc.tensor.matmul(out=h_ps[:, :], lhsT=w1t[:, :],
                             rhs=snrl[:, :], start=True, stop=True)
            hr = pconst.tile([20, B_LOC], f32)
            nc.scalar.activation(hr[:, :], h_ps[:, :], AF.Relu, bias=b1s[:, :], scale=1.0)
            w2t = pconst.tile([20, 4], f32)
            nc.sync.dma_start(out=w2t[:, :], in_=w2_h[:, :].rearrange("a b -> b a"))
            b2s = pconst.tile([4, 1], f32)
            nc.sync.dma_start(out=b2s[:, :], in_=b2_h[:, :])
            par_ps = ppsB.tile([4, B_LOC], f32, tag="aux")
            nc.tensor.matmul(out=par_ps[:, :], lhsT=w2t[:, :],
                             rhs=hr[:, :], start=True, stop=True)
            params = pconst.tile([4, B_LOC], f32)
            nc.scalar.activation(
                params[:, :], par_ps[:, :], AF.Sigmoid, bias=b2s[:, :], scale=1.0)
            pflat = pconst.tile([1, 4 * B_LOC], f32)
            nc.sync.dma_start(
                out=pflat[0:1, :].rearrange("x (p b) -> x p b", p=4),
                in_=params[:, :])
            ones1 = pconst.tile([1, 128], f32)
            nc.vector.memset(ones1[:, :], 1.0)
            rep_ps = ppsB.tile([128, 4 * B_LOC], f32, tag="aux")
            nc.tensor.matmul(out=rep_ps[:, :], lhsT=ones1[:, :],
                             rhs=pflat[:, :], start=True, stop=True)
            sl = lambda r: rep_ps[:, r * B_LOC:(r + 1) * B_LOC]
            # per-batch coefs, broadcast to all 128 partitions
            cbeta = pconst.tile([128, B_LOC], f32)
            nc.scalar.activation(cbeta[:, :], sl(0), AF.Copy)
            cgam = pconst.tile([128, B_LOC], f32)
            nc.scalar.activation(cgam[:, :], sl(1), AF.Copy)
            c1mgam = pconst.tile([128, B_LOC], f32)
            nc.scalar.activation(c1mgam[:, :], sl(1), AF.Copy, scale=-1.0, bias=1.0)
            cwi = pconst.tile([128, B_LOC], f32)
            nc.scalar.activation(cwi[:, :], sl(2), AF.Copy, scale=1.5)
            cwe = pconst.tile([128, B_LOC], f32)
            nc.scalar.activation(cwe[:, :], sl(3), AF.Copy, scale=1.5)
            cgwe = pconst.tile([128, B_LOC], f32)
            nc.vector.tensor_tensor(
                out=cgwe[:, :], in0=cgam[:, :], in1=cwe[:, :], op=OP.mult)
            cwa = pconst.tile([128, B_LOC], f32)  # Wi*(1-beta)
            nc.scalar.activation(cwa[:, :], sl(0), AF.Copy, scale=-1.0, bias=1.0)
            nc.vector.tensor_tensor(
                out=cwa[:, :], in0=cwa[:, :], in1=cwi[:, :], op=OP.mult)
            cwb = pconst.tile([128, B_LOC], f32)  # Wi*beta
            nc.vector.tensor_tensor(
                out=cwb[:, :], in0=cwi[:, :], in1=cbeta[:, :], op=OP.mult)
            cgwi = pconst.tile([128, B_LOC], f32)  # gamma*Wi
            nc.vector.tensor_tensor(
                out=cgwi[:, :], in0=cgam[:, :], in1=cwi[:, :], op=OP.mult)
            cgwa = pconst.tile([128, B_LOC], f32)  # gamma*Wi*(1-beta)
            nc.vector.tensor_tensor(
                out=cgwa[:, :], in0=cgam[:, :], in1=cwa[:, :], op=OP.mult)
            ck = [pconst.tile([128, EF], f32, name=f"ck{i}") for i in range(2)]
            nc.scalar.dma_start(out=ck[0][:, :], in_=chnk_h[:, 0:EF])

            # ---------- main loop ----------
            def egather(dst, src_h_, idx_ap, elem, h0, nh):
                # gather chunks [h0, h0+nh) of the E-permutation into dst
                dst3 = dst[:, :].rearrange("p (c w) -> p c w", w=elem)
                for i in range(nh):
                    h = h0 + i
                    nc.gpsimd.dma_gather(
                        out_ap=dst3[:, i * (CH // 128):(i + 1) * (CH // 128), :],
                        in_ap=src_h_[:, :],
                        idxs_ap=idx_ap[:, h * (CH // 16):(h + 1) * (CH // 16)],
                        num_idxs=CH, num_idxs_reg=CH, elem_size=elem,
                        queue_num=h % NQ, single_packet=False)

            ellw = pnw.tile([128, NF], f32, tag="ellw")
            tprev = pnw.tile([128, NF], f32, tag="tprev")
            tg = pnw.tile([128, NF], f32, tag="tg")

            for tau in range(T_RRD):
                tkt = tk_sb[:, tau * EW:(tau + 1) * EW]
                cvt = cv_sb[:, tau * EW:(tau + 1) * EW]

                ckt = ck[tau % 2]
                if tau + 1 < T_RRD:
                    nc.scalar.dma_start(
                        out=ck[(tau + 1) % 2][:, :],
                        in_=chnk_h[:, (tau + 1) * EF:(tau + 2) * EF])

                v2c = pwork.tile([128, EF], f32, tag="v2c")
                c2v = pwork.tile([128, EF], bf16, tag="c2v")
                HC = EC // 2   # 64 chunks per half
                HE = EF // 2   # 1024 free per half
                NH = (E // CH) // 2  # 8 dma chunks per half

                def bch(coef):
                    return coef[:, :].rearrange("p (c b) -> p c b", c=1).to_broadcast(
                        [128, HC, B_LOC])

                for t in range(T_INNER):
                    # gather gamma*t to check-sorted edges, split in halves so
                    # half 1 drains while half 0 computes.  t_dram holds
                    # gamma*t_prev at tau start (staged by the previous
                    # iteration); at (tau=0, t=0) messages depend on chn only.
                    tk_h = [pwork.tile([128, HC * 64], f32, tag=f"tg{h}",
                                       name=f"tk{h}") for h in range(2)]
                    if not (tau == 0 and t == 0):
                        egather(tk_h[0], t_dram, tkt, 64, 0, NH)
                        egather(tk_h[1], t_dram, tkt, 64, NH, NH)
                    if t == 0:
                        # ellw = Wi*((1-beta)*chn + beta*t_prev): off the
                        # critical chain, runs while the gather drains
                        if tau == 0:
                            nc.vector.tensor_tensor(
                                out=e3(ellw[:, :]), in0=e3(chn_sb[:, :]),
                                in1=bcn(cwi), op=OP.mult)
                        else:
                            ea = pnw.tile([128, NF], f32, tag="nw")
                            nc.vector.tensor_tensor(
                                out=e3(ea[:, :]), in0=e3(chn_sb[:, :]),
                                in1=bcn(cwa), op=OP.mult)
                            nc.vector.tensor_tensor(
                                out=e3(ellw[:, :]), in0=e3(tprev[:, :]),
                                in1=bcn(cwb), op=OP.mult)
                            nc.vector.tensor_tensor(
                                out=ellw[:, :], in0=ellw[:, :], in1=ea[:, :],
                                op=OP.add)

                    # gather-independent pre-work for BOTH halves first,
                    # so no tk semaphore wait lands ahead of it in the
                    # vector stream (it fills the tk drain window)
                    c2vd = pwork.tile([128, EF], bf16, tag="c2vd")
                    for half in range(2):
                        hs = slice(half * HE, (half + 1) * HE)
                        cslc = slice(half * HC, (half + 1) * HC)
                        v2ch = e3(v2c[:, :])[:, cslc, :]
                        c2vh = e3(c2v[:, :])[:, cslc, :]
                        ckh = e3(ckt[:, :])[:, cslc, :]
                        if t == 0 and tau == 0:
                            nc.vector.tensor_tensor(
                                out=v2ch, in0=ckh, in1=bch(cgwi), op=OP.mult)
                        elif t == 0:
                            nc.vector.tensor_tensor(
                                out=v2ch, in0=ckh, in1=bch(cgwa), op=OP.mult)
                        else:
                            dd = pwork.tile([128, EF], f32, tag="dd")
                            ddh = e3(dd[:, :])[:, cslc, :]
                            nc.vector.tensor_tensor(
                                out=ddh, in0=c2vh, in1=bch(cgwe), op=OP.mult)
                            nc.vector.tensor_tensor(
                                out=e3(c2vd[:, :])[:, cslc, :], in0=c2vh,
                                in1=bch(c1mgam), op=OP.mult)
                            nc.vector.tensor_tensor(
                                out=v2ch, in0=v2ch, in1=bch(c1mgam), op=OP.mult)
                            nc.vector.tensor_tensor(
                                out=v2c[:, hs], in0=v2c[:, hs], in1=dd[:, hs],
                                op=OP.subtract)
                    for half in range(2):
                        hs = slice(half * HE, (half + 1) * HE)
                        ls = slice(EF + half * HE, EF + (half + 1) * HE)
                        cslc = slice(half * HC, (half + 1) * HC)
                        v2ch = e3(v2c[:, :])[:, cslc, :]
                        c2vh = e3(c2v[:, :])[:, cslc, :]
                        if t == 0 and tau == 0:
                            pass
                        elif t == 0:
                            dd2 = pwork.tile([128, EF], f32, tag="dd")
                            dd2h = e3(dd2[:, :])[:, cslc, :]
                            nc.vector.tensor_tensor(
                                out=dd2h, in0=sel(tk_h[half], 64), in1=bch(cwb),
                                op=OP.mult)
                            nc.vector.tensor_tensor(
                                out=v2c[:, hs], in0=v2c[:, hs], in1=dd2[:, hs],
                                op=OP.add)
                        else:
                            nc.vector.tensor_tensor(
                                out=v2ch, in0=v2ch, in1=sel(tk_h[half], 64),
                                op=OP.add)

                        # ---- H step (this half) ----
                        neg = pwork.tile([128, EF], bf16, tag="neg")
                        nc.vector.tensor_scalar(
                            out=neg[:, hs], in0=v2c[:, hs],
                            scalar1=0.0, scalar2=None, op0=OP.is_lt)
                        ab = pwork.tile([128, EF], f32, tag="ab")
                        nc.vector.tensor_scalar(
                            out=ab[:, hs].bitcast(i32), in0=v2c[:, hs].bitcast(i32),
                            scalar1=0x7FFFFFFF, scalar2=None, op0=OP.bitwise_and)
                        nc.vector.tensor_scalar(
                            out=ab[:, hs], in0=ab[:, hs],
                            scalar1=AMP_MIN, scalar2=-1.0, op0=OP.max, op1=OP.mult)
                        nc.scalar.activation(ab[:, hs], ab[:, hs], AF.Exp)
                        lnnd = pwork.tile([128, 2 * EF], bf16, tag="lnnd")
                        nc.scalar.activation(
                            lnnd[:, hs], ab[:, hs], AF.Ln, scale=-1.0, bias=1.0)
                        nc.scalar.activation(
                            lnnd[:, ls], ab[:, hs], AF.Ln, scale=1.0, bias=1.0)

                        amp_ps = ppsA.tile([128, EF], f32, tag="amp")
                        s_ps = ppsB.tile([128, EF], f32, tag="aux")
                        for i in range(HE // 512):
                            cs = slice(half * HE + i * 512, half * HE + (i + 1) * 512)
                            cl = slice(EF + half * HE + i * 512,
                                       EF + half * HE + (i + 1) * 512)
                            nc.tensor.matmul(
                                out=amp_ps[:, cs], lhsT=mloo_bf[:, :],
                                rhs=lnnd[:, cs], start=True, stop=False)
                            nc.tensor.matmul(
                                out=amp_ps[:, cs], lhsT=mloon_bf[:, :],
                                rhs=lnnd[:, cl], start=False, stop=True)
                            nc.tensor.matmul(
                                out=s_ps[:, cs], lhsT=mloo_bf[:, :],
                                rhs=neg[:, cs], start=True, stop=True)
                        g = pwork.tile([128, EF], f32, tag="g")
                        nc.scalar.activation(g[:, hs], amp_ps[:, hs], AF.Exp)
                        # parity sign runs in parallel with the Exp/Ln chain
                        # (atanh is odd, so the sign applies to lr directly)
                        si = pwork.tile([128, EF], i32, tag="dd")
                        nc.vector.tensor_copy(si[:, hs], s_ps[:, hs])
                        nc.vector.tensor_scalar(
                            out=si[:, hs], in0=si[:, hs],
                            scalar1=1, scalar2=None, op0=OP.bitwise_and)
                        sg = pwork.tile([128, EF], f32, tag="ab")
                        nc.vector.tensor_scalar(
                            out=sg[:, hs], in0=si[:, hs],
                            scalar1=-2.0, scalar2=1.0, op0=OP.mult, op1=OP.add)
                        nc.vector.tensor_tensor(
                            out=e3(sg[:, :])[:, cslc, :],
                            in0=e3(sg[:, :])[:, cslc, :], in1=bch(cgam),
                            op=OP.mult)
                        ln2 = pwork.tile([128, 2 * EF], bf16, tag="lnnd")
                        nc.scalar.activation(
                            ln2[:, hs], g[:, hs], AF.Ln, scale=EPS1, bias=1.0)
                        nc.scalar.activation(
                            ln2[:, ls], g[:, hs], AF.Ln, scale=-EPS1, bias=1.0)
                        lr = pwork.tile([128, EF], f32, tag="neg")
                        nc.vector.tensor_tensor(
                            out=lr[:, hs], in0=ln2[:, hs], in1=ln2[:, ls],
                            op=OP.subtract)
                        if t == 0:
                            nc.vector.tensor_tensor(
                                out=c2v[:, hs], in0=lr[:, hs], in1=sg[:, hs],
                                op=OP.mult)
                        else:
                            nc.vector.tensor_tensor(
                                out=lr[:, hs], in0=lr[:, hs], in1=sg[:, hs],
                                op=OP.mult)
                            nc.vector.tensor_tensor(
                                out=c2v[:, hs], in0=c2vd[:, hs], in1=lr[:, hs],
                                op=OP.add)
                        # stage this half of c2v (alternate DMA engines)
                        eng = nc.sync if half == 0 else nc.scalar
                        eng.dma_start(
                            out=c2v_dram[:, 0:B_LOC].rearrange(
                                "(p c) b -> p c b", p=128)[:, cslc, :],
                            in_=c2vh)
                    # gather c2v to var-sorted order (halved)
                    cv_h = [pwork.tile([128, HC * 128], bf16, tag=f"tg{h}",
                                       name=f"cv{h}") for h in range(2)]
                    egather(cv_h[0], c2v_dram, cvt, 128, 0, NH)
                    egather(cv_h[1], c2v_dram, cvt, 128, NH, NH)
                    s_out = ppsA.tile([128, NF], f32, tag="amp")
                    for half in range(2):
                        cvv_r = cv_h[half][:, :].rearrange(
                            "p (cp r w) -> p r cp w", r=DV, w=128)
                        oslc = e3(s_out[:, :])[:, half * 16:(half + 1) * 16, :]
                        for r in range(DV):
                            nc.tensor.matmul(
                                out=oslc,
                                lhsT=rmat_bf[:, r * 128:(r + 1) * 128],
                                rhs=cvv_r[:, r, :, 0:B_LOC],
                                start=(r == 0), stop=(r == DV - 1))
                    # t_new = ellw + We * colsum ; stage gamma*t_new ; output
                    tnew = pnw.tile([128, NF], f32, tag="tprev")
                    nc.vector.tensor_tensor(
                        out=e3(tnew[:, :]), in0=e3(s_out[:, :]), in1=bcn(cwe),
                        op=OP.mult)
                    nc.vector.tensor_tensor(
                        out=tnew[:, :], in0=tnew[:, :], in1=ellw[:, :], op=OP.add)
                    if t < T_INNER - 1 or tau < T_RRD - 1:
                        nc.vector.tensor_tensor(
                            out=e3(tg[:, :]), in0=e3(tnew[:, :]), in1=bcn(cgam),
                            op=OP.mult)
                        nc.sync.dma_start(
                            out=t_dram[:, 0:B_LOC].rearrange(
                                "(p c) b -> p c b", p=128),
                            in_=e3(tg[:, :]))
                    so = (tau * T_INNER + t) * N
                    nc.scalar.dma_start(
                        out=out_h[so:so + N, :].rearrange("(c p) b -> p c b", p=128),
                        in_=e3(tnew[:, :]))
                    tprev = tnew
    return nc


def _nrow(n):
    n = np.asarray(n)
    return ((n % 128) * NC_ + n // 128).astype(np.int64)


def _erow(k):
    k = np.asarray(k)
    return ((k % 128) * EC + k // 128).astype(np.int64)


def _wrap16(lin):
    lin = np.asarray(lin)
    n = lin.shape[0]
    w = np.zeros((128, n // 16), np.int16)
    idx = np.arange(n)
    w[idx % 16, idx // 16] = lin.astype(np.int16)
    for g in range(1, 8):
        w[g * 16:(g + 1) * 16, :] = w[0:16, :]
    return np.ascontiguousarray(w)


def _prep_tables(row_idx, col_idx, perms, inv_perms):
    row_idx = np.asarray(row_idx)
    perms = np.asarray(perms)
    inv_perms = np.asarray(inv_perms)
    sigma = np.argsort(row_idx, kind="stable")  # cs pos k -> orig edge
    inv_sigma = np.argsort(sigma)               # orig edge -> cs pos
    ks = np.arange(E)
    tks, cvs = [], []
    for tau in range(T_RRD):
        # tk: for cs pos k, node row of physical var of that edge
        w_of_cs = perms[tau][sigma // DV]          # physical var id
        tks.append(_wrap16(_nrow(w_of_cs)))
        # cv: for var-sorted pos q (q = 4*w + i), cs row of that edge
        e_of_q = 4 * inv_perms[tau][ks // DV] + ks % DV
        cvs.append(_wrap16(_erow(inv_sigma[e_of_q])))
    tk16 = np.concatenate(tks, axis=1)
    cv16 = np.concatenate(cvs, axis=1)
    w_cs_all = np.stack([perms[tau][sigma // DV] for tau in range(T_RRD)])

    mloo = np.zeros((128, 128), np.float32)
    for tt in range(128 // DC):
        mloo[tt * DC:(tt + 1) * DC, tt * DC:(tt + 1) * DC] = 1.0
    mloo -= np.eye(128, dtype=np.float32)
    rmat = np.zeros((128, 4 * 128), np.float32)
    for r in range(DV):
        for p in range(128):
            rmat[p, r * 128 + 32 * r + p // DV] = 1.0
    return dict(tk16=tk16, cv16=cv16, mloo=mloo, mloon=-mloo, rmat=rmat), w_cs_all


def _make_in_maps(chn_llr, W1, b1, W2, b2, row_idx, col_idx, perms, inv_perms):
    tables, w_cs_all = _prep_tables(row_idx, col_idx, perms, inv_perms)
    chn = np.asarray(chn_llr, np.float32)
    common = {
        "W1": np.asarray(W1, np.float32).reshape(20, 1),
        "b1": np.asarray(b1, np.float32).reshape(20, 1),
        "W2": np.asarray(W2, np.float32).reshape(4, 20),
        "b2": np.asarray(b2, np.float32).reshape(4, 1),
        **tables,
    }
    # chn tile [128, NC_, B_LOC]: tile[p, c] = chn[c*128 + p]
    chn_t = chn.reshape(NC_, 128, B_FULL).transpose(1, 0, 2)  # [p, c, B]
    # chnk: per tau, chn gathered to check-sorted edge positions
    # [T, E, B] -> per tau tile [128, EC, B_LOC]
    chnk_full = chn[w_cs_all]  # [T, E, B]
    chnk_t = chnk_full.reshape(T_RRD, EC, 128, B_FULL).transpose(0, 2, 1, 3)
    in_maps = []
    for c in range(N_CORES):
        m = dict(common)
        m["chn"] = np.ascontiguousarray(
            chn_t[:, :, c * B_LOC:(c + 1) * B_LOC].reshape(128, NF))
        m["chnk"] = np.ascontiguousarray(
            chnk_t[:, :, :, c * B_LOC:(c + 1) * B_LOC]
            .transpose(1, 0, 2, 3).reshape(128, T_RRD * EF))
        in_maps.append(m)
    return in_maps


def kernel(chn_llr, W1, b1, W2, b2, row_idx, col_idx, perms, inv_perms):
    from concourse.bass_utils import run_bass_kernel_spmd

    if "nc" not in _CACHE:
        nc = _build_nc()
        nc.finalize()
        _CACHE["nc"] = nc
    nc = _CACHE["nc"]

    in_maps = _make_in_maps(
        chn_llr, W1, b1, W2, b2, row_idx, col_idx, perms, inv_perms)
    res = run_bass_kernel_spmd(nc, in_maps, core_ids=list(range(N_CORES)))
    outs = [res.results[c]["out"].reshape(T_RRD, T_INNER, N, B_LOC)
            for c in range(N_CORES)]
    return np.concatenate(outs, axis=3).astype(np.float32)


# revision 11
# speedup vs baseline: 1.1427x; 1.1427x over previous
"""Trainium2 Bass kernel for nn_AdaRRD_Decoder — physical-domain restructure.

Strategy
--------
Data-parallel over batch: B=128 split 16 per core.  All node-space tensors are
keyed by PHYSICAL variable id, which folds the per-tau automorphism perms into
the (compile-time known) edge routing tables:
  - soft mixing, output, and the tau->tau carry become pure elementwise ops;
  - per inner iteration only TWO irregular gathers remain:
      tk : gamma-scaled node tensor t -> check-sorted edge positions
      cv : damped C2V messages (check-sorted) -> var-sorted edge positions
    Each is 16 dma_gather chunks spread round-robin over 4 SWDGE queues
    (parallel descriptor rings ~3x the single-ring drain rate).
Check-side leave-one-out sums are (J8-I8) block-diag matmuls on TensorE in the
check-sorted layout; var-side column sums are fixed rmat matmuls in the
var-sorted layout.  Transcendentals use only Exp/Ln (one ACT table set):
  log(tanh(x/2)) = log(1-e^-x) - log(1+e^-x)
  2*atanh(y)     = log(1+y) - log(1-y)
"""

import numpy as np

N = 4096
M = 2048
DV = 4
DC = 8
E = N * DV
B_FULL = 128
N_CORES = 8
B_LOC = B_FULL // N_CORES  # 16
T_RRD = 10
T_INNER = 2
LLR_CLIP = 15.0
AMP_MIN = float(-np.log(np.tanh(LLR_CLIP / 2.0)))
TH_MIN = float(np.tanh(-np.log(np.tanh(LLR_CLIP / 2.0)) / 2.0))
EPS1 = 1.0 - 1e-6
LOG10E_10 = float(10.0 / np.log(10.0))

EC = E // 128   # 128 edge chunks
NC_ = N // 128  # 32 var chunks
EF = EC * B_LOC   # 2048
NF = NC_ * B_LOC  # 512
CH = 1024
NQ = 4  # SWDGE queues

_CACHE = {}


def _build_nc():
    import concourse.bass as bass
    import concourse.bacc as bacc
    import concourse.tile as tile
    from concourse import mybir

    f32 = mybir.dt.float32
    bf16 = mybir.dt.bfloat16
    i32 = mybir.dt.int32
    i16 = mybir.dt.int16
    AF = mybir.ActivationFunctionType
    OP = mybir.AluOpType

    nc = bacc.Bacc(trn_type="TRN2", dynamic_dma_scratch_size=32768,
                   num_swdge_queues=NQ)

    # ---- external I/O ----
    chn_h = nc.declare_dram_parameter("chn", [128, NF], f32, isOutput=False)
    w1_h = nc.declare_dram_parameter("W1", [20, 1], f32, isOutput=False)
    b1_h = nc.declare_dram_parameter("b1", [20, 1], f32, isOutput=False)
    w2_h = nc.declare_dram_parameter("W2", [4, 20], f32, isOutput=False)
    b2_h = nc.declare_dram_parameter("b2", [4, 1], f32, isOutput=False)
    EW = E // 16  # 1024 idx cols per tau
    tk_h = nc.declare_dram_parameter("tk16", [128, T_RRD * EW], i16, isOutput=False)
    cv_h = nc.declare_dram_parameter("cv16", [128, T_RRD * EW], i16, isOutput=False)
    chnk_h = nc.declare_dram_parameter(
        "chnk", [128, T_RRD * EF], f32, isOutput=False)
    mloo_h = nc.declare_dram_parameter("mloo", [128, 128], f32, isOutput=False)
    mloon_h = nc.declare_dram_parameter("mloon", [128, 128], f32, isOutput=False)
    rmat_h = nc.declare_dram_parameter("rmat", [128, 512], f32, isOutput=False)
    out_h = nc.declare_dram_parameter(
        "out", [T_RRD * T_INNER * N, B_LOC], f32, isOutput=True)

    # DRAM staging (rows padded to 256 B for gather elem granularity)
    t_dram = nc.dram_tensor("t_dram", [N, 64], f32)
    c2v_dram = nc.dram_tensor("c2v_dram", [E, 128], bf16)

    def bce(coef):
        return coef[:, :].rearrange("p (c b) -> p c b", c=1).to_broadcast(
            [128, EC, B_LOC])

    def bcn(coef):
        return coef[:, :].rearrange("p (c b) -> p c b", c=1).to_broadcast(
            [128, NC_, B_LOC])

    def e3(ap):
        return ap.rearrange("p (c b) -> p c b", b=B_LOC)

    def sel(gdst, w):
        return gdst[:, :].rearrange("p (c w) -> p c w", w=w)[:, :, 0:B_LOC]

    with tile.TileContext(nc) as tc:
        with (
            tc.tile_pool(name="const", bufs=1) as pconst,
            tc.tile_pool(name="work", bufs=1) as pwork,
            tc.tile_pool(name="nwork", bufs=1) as pnw,
            tc.tile_pool(name="psA", bufs=1, space="PSUM") as ppsA,
            tc.tile_pool(name="psB", bufs=1, space="PSUM") as ppsB,
        ):
            # ---------- constants ----------
            tk_sb = pconst.tile([128, T_RRD * EW], i16)
            nc.sync.dma_start(out=tk_sb[:, :], in_=tk_h[:, :])
            cv_sb = pconst.tile([128, T_RRD * EW], i16)
            nc.sync.dma_start(out=cv_sb[:, :], in_=cv_h[:, :])
            # zero padded staging (pad cols are gathered; must be defined)
            zt = pconst.tile([128, 512], f32)
            nc.vector.memset(zt[:, :], 0.0)
            zf = zt[:, :].rearrange("p (a b) -> p a b", b=64)
            ztb = zt[:, :].bitcast(bf16).rearrange("p (a b) -> p a b", b=128)
            for z in range(4):
                nc.sync.dma_start(
                    out=t_dram[z * N // 4:(z + 1) * N // 4, :].rearrange(
                        "(p a) b -> p a b", p=128), in_=zf)
            for z in range(16):
                nc.sync.dma_start(
                    out=c2v_dram[z * E // 16:(z + 1) * E // 16, :].rearrange(
                        "(p a) b -> p a b", p=128), in_=ztb)
            mloo_sb = pconst.tile([128, 128], f32)
            nc.sync.dma_start(out=mloo_sb[:, :], in_=mloo_h[:, :])
            mloon_sb = pconst.tile([128, 128], f32)
            nc.sync.dma_start(out=mloon_sb[:, :], in_=mloon_h[:, :])
            rmat_sb = pconst.tile([128, 512], f32)
            nc.sync.dma_start(out=rmat_sb[:, :], in_=rmat_h[:, :])
            mloo_bf = pconst.tile([128, 128], bf16)
            nc.vector.tensor_copy(mloo_bf[:, :], mloo_sb[:, :])
            mloon_bf = pconst.tile([128, 128], bf16)
            nc.vector.tensor_copy(mloon_bf[:, :], mloon_sb[:, :])
            rmat_bf = pconst.tile([128, 512], bf16)
            nc.vector.tensor_copy(rmat_bf[:, :], rmat_sb[:, :])

            chn_sb = pconst.tile([128, NF], f32)
            nc.sync.dma_start(out=chn_sb[:, :], in_=chn_h[:, :])

            # ---------- adapter NN ----------
            sq = pnw.tile([128, NF], f32, tag="nw")
            nc.scalar.activation(sq[:, :], chn_sb[:, :], AF.Square)
            ones128 = pconst.tile([128, 1], f32)
            nc.vector.memset(ones128[:, :], 1.0)
            e1_ps = ppsB.tile([1, NF], f32, tag="aux")
            nc.tensor.matmul(out=e1_ps[:, :], lhsT=ones128[:, :],
                             rhs=sq[:, :], start=True, stop=True)
            e1_sb = pconst.tile([1, NF], f32)
            nc.vector.tensor_copy(e1_sb[:, :], e1_ps[:, :])
            e32 = pconst.tile([NC_, B_LOC], f32)
            nc.sync.dma_start(
                out=e32[:, :],
                in_=e1_sb[0:1, :].rearrange("x (c b) -> x c b", c=NC_))
            ones32 = pconst.tile([NC_, 1], f32)
            nc.vector.memset(ones32[:, :], 1.0)
            est_ps = ppsB.tile([1, B_LOC], f32, tag="aux")
            nc.tensor.matmul(out=est_ps[:, :], lhsT=ones32[:, :],
                             rhs=e32[:, :], start=True, stop=True)
            est = pconst.tile([1, B_LOC], f32)
            nc.scalar.activation(est[:, :], est_ps[:, :], AF.Copy, scale=1.0 / N)
            s1 = pconst.tile([1, B_LOC], f32)
            nc.scalar.activation(s1[:, :], est_ps[:, :], AF.Ln, scale=1.0 / N, bias=1.0)
            s2 = pconst.tile([1, B_LOC], f32)
            nc.scalar.activation(s2[:, :], s1[:, :], AF.Exp, scale=0.5)
            nc.vector.tensor_scalar(
                out=s2[:, :], in0=s2[:, :], scalar1=1.0, scalar2=None, op0=OP.add)
            nc.vector.reciprocal(s2[:, :], s2[:, :])
            q = pconst.tile([1, B_LOC], f32)
            nc.vector.tensor_tensor(out=q[:, :], in0=est[:, :], in1=s2[:, :], op=OP.mult)
            snrl = pconst.tile([1, B_LOC], f32)
            nc.scalar.activation(snrl[:, :], q[:, :], AF.Ln, scale=0.5)

            w1t = pconst.tile([1, 20], f32)
            nc.sync.dma_start(out=w1t[:, :], in_=w1_h[:, :].rearrange("a b -> b a"))
            nc.scalar.activation(w1t[:, :], w1t[:, :], AF.Copy, scale=LOG10E_10)
            b1s = pconst.tile([20, 1], f32)
            nc.sync.dma_start(out=b1s[:, :], in_=b1_h[:, :])
            h_ps = ppsB.tile([20, B_LOC], f32, tag="aux")
            nc.tensor.matmul(out=h_ps[:, :], lhsT=w1t[:, :],
                             rhs=snrl[:, :], start=True, stop=True)
            hr = pconst.tile([20, B_LOC], f32)
            nc.scalar.activation(hr[:, :], h_ps[:, :], AF.Relu, bias=b1s[:, :], scale=1.0)
            w2t = pconst.tile([20, 4], f32)
            nc.sync.dma_start(out=w2t[:, :], in_=w2_h[:, :].rearrange("a b -> b a"))
            b2s = pconst.tile([4, 1], f32)
            nc.sync.dma_start(out=b2s[:, :], in_=b2_h[:, :])
            par_ps = ppsB.tile([4, B_LOC], f32, tag="aux")
            nc.tensor.matmul(out=par_ps[:, :], lhsT=w2t[:, :],
                             rhs=hr[:, :], start=True, stop=True)
            params = pconst.tile([4, B_LOC], f32)
            nc.scalar.activation(
                params[:, :], par_ps[:, :], AF.Sigmoid, bias=b2s[:, :], scale=1.0)
            pflat = pconst.tile([1, 4 * B_LOC], f32)
            nc.sync.dma_start(
                out=pflat[0:1, :].rearrange("x (p b) -> x p b", p=4),
                in_=params[:, :])
            ones1 = pconst.tile([1, 128], f32)
            nc.vector.memset(ones1[:, :], 1.0)
            rep_ps = ppsB.tile([128, 4 * B_LOC], f32, tag="aux")
            nc.tensor.matmul(out=rep_ps[:, :], lhsT=ones1[:, :],
                             rhs=pflat[:, :], start=True, stop=True)
            sl = lambda r: rep_ps[:, r * B_LOC:(r + 1) * B_LOC]
            # per-batch coefs, broadcast to all 128 partitions
            cbeta = pconst.tile([128, B_LOC], f32)
            nc.scalar.activation(cbeta[:, :], sl(0), AF.Copy)
            cgam = pconst.tile([128, B_LOC], f32)
            nc.scalar.activation(cgam[:, :], sl(1), AF.Copy)
            c1mgam = pconst.tile([128, B_LOC], f32)
            nc.scalar.activation(c1mgam[:, :], sl(1), AF.Copy, scale=-1.0, bias=1.0)
            cwi = pconst.tile([128, B_LOC], f32)
            nc.scalar.activation(cwi[:, :], sl(2), AF.Copy, scale=1.5)
            cwe = pconst.tile([128, B_LOC], f32)
            nc.scalar.activation(cwe[:, :], sl(3), AF.Copy, scale=1.5)
            cgwe = pconst.tile([128, B_LOC], f32)
            nc.vector.tensor_tensor(
                out=cgwe[:, :], in0=cgam[:, :], in1=cwe[:, :], op=OP.mult)
            cwa = pconst.tile([128, B_LOC], f32)  # Wi*(1-beta)
            nc.scalar.activation(cwa[:, :], sl(0), AF.Copy, scale=-1.0, bias=1.0)
            nc.vector.tensor_tensor(
                out=cwa[:, :], in0=cwa[:, :], in1=cwi[:, :], op=OP.mult)
            cwb = pconst.tile([128, B_LOC], f32)  # Wi*beta
            nc.vector.tensor_tensor(
                out=cwb[:, :], in0=cwi[:, :], in1=cbeta[:, :], op=OP.mult)
            cgwi = pconst.tile([128, B_LOC], f32)  # gamma*Wi
            nc.vector.tensor_tensor(
                out=cgwi[:, :], in0=cgam[:, :], in1=cwi[:, :], op=OP.mult)
            cgwa = pconst.tile([128, B_LOC], f32)  # gamma*Wi*(1-beta)
            nc.vector.tensor_tensor(
                out=cgwa[:, :], in0=cgam[:, :], in1=cwa[:, :], op=OP.mult)
            ck = [pconst.tile([128, EF], f32, name=f"ck{i}") for i in range(2)]
            nc.scalar.dma_start(out=ck[0][:, :], in_=chnk_h[:, 0:EF])

            # ---------- main loop ----------
            def egather(dst, src_h_, idx_ap, elem, h0, nh):
                # gather chunks [h0, h0+nh) of the E-permutation into dst
                dst3 = dst[:, :].rearrange("p (c w) -> p c w", w=elem)
                for i in range(nh):
                    h = h0 + i
                    nc.gpsimd.dma_gather(
                        out_ap=dst3[:, i * (CH // 128):(i + 1) * (CH // 128), :],
                        in_ap=src_h_[:, :],
                        idxs_ap=idx_ap[:, h * (CH // 16):(h + 1) * (CH // 16)],
                        num_idxs=CH, num_idxs_reg=CH, elem_size=elem,
                        queue_num=h % NQ, single_packet=False)

            ellw = pnw.tile([128, NF], f32, tag="ellw")
            tprev = pnw.tile([128, NF], f32, tag="tprev")
            tg = pnw.tile([128, NF], f32, tag="tg")

            for tau in range(T_RRD):
                tkt = tk_sb[:, tau * EW:(tau + 1) * EW]
                cvt = cv_sb[:, tau * EW:(tau + 1) * EW]

                ckt = ck[tau % 2]
                if tau + 1 < T_RRD:
                    nc.scalar.dma_start(
                        out=ck[(tau + 1) % 2][:, :],
                        in_=chnk_h[:, (tau + 1) * EF:(tau + 2) * EF])

                v2c = pwork.tile([128, EF], f32, tag="v2c")
                c2v = pwork.tile([128, EF], bf16, tag="c2v")
                HC = EC // 2   # 64 chunks per half
                HE = EF // 2   # 1024 free per half
                NH = (E // CH) // 2  # 8 dma chunks per half

                def bch(coef):
                    return coef[:, :].rearrange("p (c b) -> p c b", c=1).to_broadcast(
                        [128, HC, B_LOC])

                for t in range(T_INNER):
                    # gather gamma*t to check-sorted edges, split in halves so
                    # half 1 drains while half 0 computes.  t_dram holds
                    # gamma*t_prev at tau start (staged by the previous
                    # iteration); at (tau=0, t=0) messages depend on chn only.
                    tk_h = [pwork.tile([128, HC * 64], f32, tag=f"tg{h}",
                                       name=f"tk{h}") for h in range(2)]
                    if not (tau == 0 and t == 0):
                        egather(tk_h[0], t_dram, tkt, 64, 0, NH)
                        egather(tk_h[1], t_dram, tkt, 64, NH, NH)
                    if t == 0:
                        # ellw = Wi*((1-beta)*chn + beta*t_prev): off the
                        # critical chain, runs while the gather drains
                        if tau == 0:
                            nc.vector.tensor_tensor(
                                out=e3(ellw[:, :]), in0=e3(chn_sb[:, :]),
                                in1=bcn(cwi), op=OP.mult)
                        else:
                            ea = pnw.tile([128, NF], f32, tag="nw")
                            nc.vector.tensor_tensor(
                                out=e3(ea[:, :]), in0=e3(chn_sb[:, :]),
                                in1=bcn(cwa), op=OP.mult)
                            nc.vector.tensor_tensor(
                                out=e3(ellw[:, :]), in0=e3(tprev[:, :]),
                                in1=bcn(cwb), op=OP.mult)
                            nc.vector.tensor_tensor(
                                out=ellw[:, :], in0=ellw[:, :], in1=ea[:, :],
                                op=OP.add)

                    for half in range(2):
                        hs = slice(half * HE, (half + 1) * HE)
                        ls = slice(EF + half * HE, EF + (half + 1) * HE)
                        cslc = slice(half * HC, (half + 1) * HC)
                        v2ch = e3(v2c[:, :])[:, cslc, :]
                        c2vh = e3(c2v[:, :])[:, cslc, :]
                        ckh = e3(ckt[:, :])[:, cslc, :]
                        if t == 0 and tau == 0:
                            nc.vector.tensor_tensor(
                                out=v2ch, in0=ckh, in1=bch(cgwi), op=OP.mult)
                        elif t == 0:
                            dd = pwork.tile([128, EF], f32, tag="dd")
                            ddh = e3(dd[:, :])[:, cslc, :]
                            nc.vector.tensor_tensor(
                                out=ddh, in0=sel(tk_h[half], 64), in1=bch(cwb),
                                op=OP.mult)
                            nc.vector.tensor_tensor(
                                out=v2ch, in0=ckh, in1=bch(cgwa), op=OP.mult)
                            nc.vector.tensor_tensor(
                                out=v2c[:, hs], in0=v2c[:, hs], in1=dd[:, hs],
                                op=OP.add)
                        else:
                            # pre = (1-gamma)*v2c - gamma*We*c2v and
                            # c2vd = (1-gamma)*c2v need no tk: computed while
                            # the gather drains
                            dd = pwork.tile([128, EF], f32, tag="dd")
                            ddh = e3(dd[:, :])[:, cslc, :]
                            nc.vector.tensor_tensor(
                                out=ddh, in0=c2vh, in1=bch(cgwe), op=OP.mult)
                            c2vd = pwork.tile([128, EF], bf16, tag="c2vd")
                            nc.vector.tensor_tensor(
                                out=e3(c2vd[:, :])[:, cslc, :], in0=c2vh,
                                in1=bch(c1mgam), op=OP.mult)
                            nc.vector.tensor_tensor(
                                out=v2ch, in0=v2ch, in1=bch(c1mgam), op=OP.mult)
                            nc.vector.tensor_tensor(
                                out=v2c[:, hs], in0=v2c[:, hs], in1=dd[:, hs],
                                op=OP.subtract)
                            nc.vector.tensor_tensor(
                                out=v2ch, in0=v2ch, in1=sel(tk_h[half], 64),
                                op=OP.add)

                        # ---- H step (this half) ----
                        neg = pwork.tile([128, EF], bf16, tag="neg")
                        nc.vector.tensor_scalar(
                            out=neg[:, hs], in0=v2c[:, hs],
                            scalar1=0.0, scalar2=None, op0=OP.is_lt)
                        ab = pwork.tile([128, EF], f32, tag="ab")
                        nc.vector.tensor_scalar(
                            out=ab[:, hs].bitcast(i32), in0=v2c[:, hs].bitcast(i32),
                            scalar1=0x7FFFFFFF, scalar2=None, op0=OP.bitwise_and)
                        nc.vector.tensor_scalar(
                            out=ab[:, hs], in0=ab[:, hs],
                            scalar1=AMP_MIN, scalar2=-1.0, op0=OP.max, op1=OP.mult)
                        nc.scalar.activation(ab[:, hs], ab[:, hs], AF.Exp)
                        lnnd = pwork.tile([128, 2 * EF], bf16, tag="lnnd")
                        nc.scalar.activation(
                            lnnd[:, hs], ab[:, hs], AF.Ln, scale=-1.0, bias=1.0)
                        nc.scalar.activation(
                            lnnd[:, ls], ab[:, hs], AF.Ln, scale=1.0, bias=1.0)

                        amp_ps = ppsA.tile([128, EF], f32, tag="amp")
                        s_ps = ppsB.tile([128, EF], f32, tag="aux")
                        for i in range(HE // 512):
                            cs = slice(half * HE + i * 512, half * HE + (i + 1) * 512)
                            cl = slice(EF + half * HE + i * 512,
                                       EF + half * HE + (i + 1) * 512)
                            nc.tensor.matmul(
                                out=amp_ps[:, cs], lhsT=mloo_bf[:, :],
                                rhs=lnnd[:, cs], start=True, stop=False)
                            nc.tensor.matmul(
                                out=amp_ps[:, cs], lhsT=mloon_bf[:, :],
                                rhs=lnnd[:, cl], start=False, stop=True)
                            nc.tensor.matmul(
                                out=s_ps[:, cs], lhsT=mloo_bf[:, :],
                                rhs=neg[:, cs], start=True, stop=True)
                        g = pwork.tile([128, EF], f32, tag="g")
                        nc.scalar.activation(g[:, hs], amp_ps[:, hs], AF.Exp)
                        # parity sign runs in parallel with the Exp/Ln chain
                        # (atanh is odd, so the sign applies to lr directly)
                        si = pwork.tile([128, EF], i32, tag="dd")
                        nc.vector.tensor_copy(si[:, hs], s_ps[:, hs])
                        nc.vector.tensor_scalar(
                            out=si[:, hs], in0=si[:, hs],
                            scalar1=1, scalar2=None, op0=OP.bitwise_and)
                        sg = pwork.tile([128, EF], f32, tag="ab")
                        nc.vector.tensor_scalar(
                            out=sg[:, hs], in0=si[:, hs],
                            scalar1=-2.0, scalar2=1.0, op0=OP.mult, op1=OP.add)
                        nc.vector.tensor_tensor(
                            out=e3(sg[:, :])[:, cslc, :],
                            in0=e3(sg[:, :])[:, cslc, :], in1=bch(cgam),
                            op=OP.mult)
                        ln2 = pwork.tile([128, 2 * EF], bf16, tag="lnnd")
                        nc.scalar.activation(
                            ln2[:, hs], g[:, hs], AF.Ln, scale=EPS1, bias=1.0)
                        nc.scalar.activation(
                            ln2[:, ls], g[:, hs], AF.Ln, scale=-EPS1, bias=1.0)
                        lr = pwork.tile([128, EF], f32, tag="neg")
                        nc.vector.tensor_tensor(
                            out=lr[:, hs], in0=ln2[:, hs], in1=ln2[:, ls],
                            op=OP.subtract)
                        if t == 0:
                            nc.vector.tensor_tensor(
                                out=c2v[:, hs], in0=lr[:, hs], in1=sg[:, hs],
                                op=OP.mult)
                        else:
                            nc.vector.tensor_tensor(
                                out=lr[:, hs], in0=lr[:, hs], in1=sg[:, hs],
                                op=OP.mult)
                            nc.vector.tensor_tensor(
                                out=c2v[:, hs], in0=c2vd[:, hs], in1=lr[:, hs],
                                op=OP.add)
                        # stage this half of c2v (alternate DMA engines)
                        eng = nc.sync if half == 0 else nc.scalar
                        eng.dma_start(
                            out=c2v_dram[:, 0:B_LOC].rearrange(
                                "(p c) b -> p c b", p=128)[:, cslc, :],
                            in_=c2vh)
                    # gather c2v to var-sorted order (halved)
                    cv_h = [pwork.tile([128, HC * 128], bf16, tag=f"tg{h}",
                                       name=f"cv{h}") for h in range(2)]
                    egather(cv_h[0], c2v_dram, cvt, 128, 0, NH)
                    egather(cv_h[1], c2v_dram, cvt, 128, NH, NH)
                    s_out = ppsA.tile([128, NF], f32, tag="amp")
                    for half in range(2):
                        cvv_r = cv_h[half][:, :].rearrange(
                            "p (cp r w) -> p r cp w", r=DV, w=128)
                        oslc = e3(s_out[:, :])[:, half * 16:(half + 1) * 16, :]
                        for r in range(DV):
                            nc.tensor.matmul(
                                out=oslc,
                                lhsT=rmat_bf[:, r * 128:(r + 1) * 128],
                                rhs=cvv_r[:, r, :, 0:B_LOC],
                                start=(r == 0), stop=(r == DV - 1))
                    # t_new = ellw + We * colsum ; stage gamma*t_new ; output
                    tnew = pnw.tile([128, NF], f32, tag="tprev")
                    nc.vector.tensor_tensor(
                        out=e3(tnew[:, :]), in0=e3(s_out[:, :]), in1=bcn(cwe),
                        op=OP.mult)
                    nc.vector.tensor_tensor(
                        out=tnew[:, :], in0=tnew[:, :], in1=ellw[:, :], op=OP.add)
                    if t < T_INNER - 1 or tau < T_RRD - 1:
                        nc.vector.tensor_tensor(
                            out=e3(tg[:, :]), in0=e3(tnew[:, :]), in1=bcn(cgam),
                            op=OP.mult)
                        nc.sync.dma_start(
                            out=t_dram[:, 0:B_LOC].rearrange(
                                "(p c) b -> p c b", p=128),
                            in_=e3(tg[:, :]))
                    so = (tau * T_INNER + t) * N
                    nc.scalar.dma_start(
                        out=out_h[so:so + N, :].rearrange("(c p) b -> p c b", p=128),
                        in_=e3(tnew[:, :]))
                    tprev = tnew
    return nc


def _nrow(n):
    n = np.asarray(n)
    return ((n % 128) * NC_ + n // 128).astype(np.int64)


def _erow(k):
    k = np.asarray(k)
    return ((k % 128) * EC + k // 128).astype(np.int64)


def _wrap16(lin):
    lin = np.asarray(lin)
    n = lin.shape[0]
    w = np.zeros((128, n // 16), np.int16)
    idx = np.arange(n)
    w[idx % 16, idx // 16] = lin.astype(np.int16)
    for g in range(1, 8):
        w[g * 16:(g + 1) * 16, :] = w[0:16, :]
    return np.ascontiguousarray(w)


def _prep_tables(row_idx, col_idx, perms, inv_perms):
    row_idx = np.asarray(row_idx)
    perms = np.asarray(perms)
    inv_perms = np.asarray(inv_perms)
    sigma = np.argsort(row_idx, kind="stable")  # cs pos k -> orig edge
    inv_sigma = np.argsort(sigma)               # orig edge -> cs pos
    ks = np.arange(E)
    tks, cvs = [], []
    for tau in range(T_RRD):
        # tk: for cs pos k, node row of physical var of that edge
        w_of_cs = perms[tau][sigma // DV]          # physical var id
        tks.append(_wrap16(_nrow(w_of_cs)))
        # cv: for var-sorted pos q (q = 4*w + i), cs row of that edge
        e_of_q = 4 * inv_perms[tau][ks // DV] + ks % DV
        cvs.append(_wrap16(_erow(inv_sigma[e_of_q])))
    tk16 = np.concatenate(tks, axis=1)
    cv16 = np.concatenate(cvs, axis=1)
    w_cs_all = np.stack([perms[tau][sigma // DV] for tau in range(T_RRD)])

    mloo = np.zeros((128, 128), np.float32)
    for tt in range(128 // DC):
        mloo[tt * DC:(tt + 1) * DC, tt * DC:(tt + 1) * DC] = 1.0
    mloo -= np.eye(128, dtype=np.float32)
    rmat = np.zeros((128, 4 * 128), np.float32)
    for r in range(DV):
        for p in range(128):
            rmat[p, r * 128 + 32 * r + p // DV] = 1.0
    return dict(tk16=tk16, cv16=cv16, mloo=mloo, mloon=-mloo, rmat=rmat), w_cs_all


def _make_in_maps(chn_llr, W1, b1, W2, b2, row_idx, col_idx, perms, inv_perms):
    tables, w_cs_all = _prep_tables(row_idx, col_idx, perms, inv_perms)
    chn = np.asarray(chn_llr, np.float32)
    common = {
        "W1": np.asarray(W1, np.float32).reshape(20, 1),
        "b1": np.asarray(b1, np.float32).reshape(20, 1),
        "W2": np.asarray(W2, np.float32).reshape(4, 20),
        "b2": np.asarray(b2, np.float32).reshape(4, 1),
        **tables,
    }
    # chn tile [128, NC_, B_LOC]: tile[p, c] = chn[c*128 + p]
    chn_t = chn.reshape(NC_, 128, B_FULL).transpose(1, 0, 2)  # [p, c, B]
    # chnk: per tau, chn gathered to check-sorted edge positions
    # [T, E, B] -> per tau tile [128, EC, B_LOC]
    chnk_full = chn[w_cs_all]  # [T, E, B]
    chnk_t = chnk_full.reshape(T_RRD, EC, 128, B_FULL).transpose(0, 2, 1, 3)
    in_maps = []
    for c in range(N_CORES):
        m = dict(common)
        m["chn"] = np.ascontiguousarray(
            chn_t[:, :, c * B_LOC:(c + 1) * B_LOC].reshape(128, NF))
        m["chnk"] = np.ascontiguousarray(
            chnk_t[:, :, :, c * B_LOC:(c + 1) * B_LOC]
            .transpose(1, 0, 2, 3).reshape(128, T_RRD * EF))
        in_maps.append(m)
    return in_maps


def kernel(chn_llr, W1, b1, W2, b2, row_idx, col_idx, perms, inv_perms):
    from concourse.bass_utils import run_bass_kernel_spmd

    if "nc" not in _CACHE:
        nc = _build_nc()
        nc.finalize()
        _CACHE["nc"] = nc
    nc = _CACHE["nc"]

    in_maps = _make_in_maps(
        chn_llr, W1, b1, W2, b2, row_idx, col_idx, perms, inv_perms)
    res = run_bass_kernel_spmd(nc, in_maps, core_ids=list(range(N_CORES)))
    outs = [res.results[c]["out"].reshape(T_RRD, T_INNER, N, B_LOC)
            for c in range(N_CORES)]
    return np.concatenate(outs, axis=3).astype(np.float32)
